# revision 1
# baseline (speedup 1.0000x reference)
"""BitLinear TRN2 kernel: out = layernorm(x) @ sign(w).T + bias.

Tensor-parallel over out_features, 8 cores: each core gets full x
[8192, 4096] + a [2048, 4096] shard of w (+ bias shard); returns the
[8192, 2048] out shard; host concats.

LN is folded around a matmul on RAW x:
    out[t,o] = (x@bw.T)[t,o] * inv_t + a_t * S[o] + bias[o]
with S[o] = sum_d bw[o,d], inv_t = 1/(std_t+eps), a_t = -mu_t*inv_t.
The rank-1 correction + bias ride the PSUM->SBUF eviction as DVE ops
against broadcast-resident S/bias rows. Stats come from bn_stats on the
natural-layout x tiles; x and sign(w) are transposed on-chip by PE
transposes (contraction dim must sit on partitions); transposed sign
weights bounce through DRAM once (one tensor per 256-wide out chunk so
every DMA is a trivial contiguous 2D pattern).

Engine discipline (walrus: fp32/f32r matmuls+transposes get ONE
semaphore-wait slot; complex multi-dim DMA APs also overflow wait
slots): sign(w) is cast to bf16 so W transposes ride the multi-wait bf16
path; every DMA feeding f32r matmuls is first "touched" by a throwaway
bf16 bitcast-transpose so the PE sequencer observes its semaphore; all
psum evictions/copies ride DVE only. Every fp32-family PE instruction
then needs at most one wait (the DVE clock).

Modes: f32r (1 cyc/row, ~2e-4 rel err), split (bf16 hi+lo, 2 matmuls,
~3e-6), bf16 (1 matmul, ~2e-3).
"""

import os
from contextlib import ExitStack

import numpy as np

import concourse.bass as bass
import concourse.tile as tile
from concourse import bacc
from concourse import mybir
from concourse.bass_utils import run_bass_kernel_spmd
from concourse.masks import make_identity
from concourse.tile_rust import add_dep_helper

F32 = mybir.dt.float32
F32R = mybir.dt.float32r
BF16 = mybir.dt.bfloat16

T, D, O_FULL, NCORES = 8192, 4096, 16384, 8
O = O_FULL // NCORES  # 2048 out-features per core
EPS = 1e-5

MODE = os.environ.get("BITLIN_MODE", "f32r")  # f32r | split | bf16

KT = D // 128  # 32 k-tiles
MC = T // 128  # 64 token chunks
SUP = 512  # tokens per superblock (resident transposed-x width)
MPS = SUP // 128  # 4 chunks per superblock
NSUP = T // SUP  # 16
KB = 8  # k-tiles per weight-prep write batch
WN = 256  # moving free width (f32r needs >=256; one PSUM bank at fp32)
NWCH = O // WN  # 8
OBP = WN // 128  # o-blocks per out chunk


def _build(mode):
    wdt = F32R if mode == "f32r" else BF16
    xdt = F32R if mode == "f32r" else BF16

    nc = bacc.Bacc("TRN2", target_bir_lowering=False, debug=False)
    x_ext = nc.declare_dram_parameter("x", [T, D], F32, isOutput=False)
    w_ext = nc.declare_dram_parameter("w", [O, D], F32, isOutput=False)
    b_ext = nc.declare_dram_parameter("b", [O], F32, isOutput=False)
    out_ext = nc.declare_dram_parameter("out", [T, O], F32, isOutput=True)
    wtq = [nc.dram_tensor(f"wtq{nw}", [128, KT, WN], wdt) for nw in range(NWCH)]
    s_d = [nc.dram_tensor(f"srow{nw}", [WN], F32) for nw in range(NWCH)]

    with tile.TileContext(nc) as tc, ExitStack() as ctx:
        singles = ctx.enter_context(tc.tile_pool(name="singles", bufs=1))
        xa_pool = ctx.enter_context(tc.tile_pool(name="xa", bufs=2))
        ws_pool = ctx.enter_context(tc.tile_pool(name="ws", bufs=1))
        xt_pool = ctx.enter_context(tc.tile_pool(name="xt", bufs=1))
        wst_pool = ctx.enter_context(tc.tile_pool(name="wst", bufs=2))
        wsb_pool = ctx.enter_context(tc.tile_pool(name="wsb", bufs=1))
        ev_pool = ctx.enter_context(tc.tile_pool(name="ev", bufs=2))
        evt_pool = ctx.enter_context(tc.tile_pool(name="evt", bufs=2))
        small = ctx.enter_context(tc.tile_pool(name="small", bufs=4))
        tmp_pool = ctx.enter_context(tc.tile_pool(name="tmp", bufs=2))
        tp_psum = ctx.enter_context(tc.tile_pool(name="tp_ps", bufs=3, space="PSUM"))
        tb_psum = ctx.enter_context(tc.tile_pool(name="tb_ps", bufs=1, space="PSUM"))
        mm_psum = ctx.enter_context(tc.tile_pool(name="mm_ps", bufs=3, space="PSUM"))
        s_psum = ctx.enter_context(tc.tile_pool(name="s_ps", bufs=1, space="PSUM"))

        identity = singles.tile([128, 128], F32)
        make_identity(nc, identity[:])
        identity_b = singles.tile([128, 128], BF16)
        nc.vector.tensor_copy(out=identity_b[:], in_=identity[:])
        ones32 = singles.tile([128, 1], F32)
        nc.vector.memset(ones32[:], 1.0)
        ones_w = singles.tile([128, 1], wdt)
        nc.vector.tensor_copy(out=ones_w[:], in_=ones32[:])
        inv_all = singles.tile([128, MC], F32)
        a_all = singles.tile([128, MC], F32)
        s_bc = singles.tile([128, NWCH, WN], BF16 if mode == "f32r" else F32)  # S bcast
        b_bc = singles.tile([128, NWCH, WN], F32)  # bias broadcast

        def touch(src_ap):
            """Throwaway bf16 transpose reading src so PE observes its sem."""
            pt = tb_psum.tile([128, 128], BF16, tag="tb")
            nc.tensor.transpose(pt[:], src_ap, identity_b[:])

        # bias broadcast (partition-stride-0 SWDGE dma)
        bap = b_ext[:]
        nc.gpsimd.dma_start(
            out=b_bc[:],
            in_=bass.AP(tensor=bap.tensor, offset=bap.offset, ap=[[0, 128]] + bap.ap),
        )

        # ------- weight prep: sign (bf16) + transpose -> per-chunk DRAM ---
        w_last_write = [None] * NWCH
        for nw in range(NWCH):
            ws_tiles = []
            for obl in range(OBP):
                ob = nw * OBP + obl
                wa = xa_pool.tile([128, D], F32, tag="xa")
                nc.gpsimd.dma_start(out=wa[:], in_=w_ext[ob * 128 : (ob + 1) * 128, :])
                ws = ws_pool.tile([128, D], BF16, tag=f"ws{obl}")
                nc.scalar.sign(out=ws[:], in_=wa[:])
                ws_tiles.append(ws)
            for kb in range(KT // KB):
                wt_sb = wsb_pool.tile([128, KB, WN], wdt, tag="wt_sb")
                for ki in range(KB):
                    k = kb * KB + ki
                    for obl in range(OBP):
                        pt = tp_psum.tile([128, 128], BF16, tag="tp")
                        nc.tensor.transpose(
                            pt[:],
                            ws_tiles[obl][:, k * 128 : (k + 1) * 128],
                            identity_b[:],
                        )
                        nc.vector.tensor_copy(
                            out=wt_sb[:, ki, obl * 128 : (obl + 1) * 128], in_=pt[:]
                        )
                inst = nc.gpsimd.dma_start(
                    out=wtq[nw][:, kb * KB : (kb + 1) * KB, :], in_=wt_sb[:]
                )
                if w_last_write[nw] is not None:
                    add_dep_helper(
                        inst.ins, w_last_write[nw].ins, sync=True, reason="wtq order"
                    )
                w_last_write[nw] = inst

        # ---------------- main: per token-superblock ---------------------
        for s in range(NSUP):
            xt = xt_pool.tile([128, KT, SUP], xdt, tag="xt_hi")
            xt_lo = (
                xt_pool.tile([128, KT, SUP], BF16, tag="xt_lo")
                if mode == "split"
                else None
            )

            for mc_i in range(MPS):
                m = s * MPS + mc_i
                xa = xa_pool.tile([128, D], F32, tag="xa")
                nc.gpsimd.dma_start(out=xa[:], in_=x_ext[m * 128 : (m + 1) * 128, :])
                touch(xa[:].bitcast(BF16)[:, 0:128])
                # --- stats ---
                st = small.tile([128, 8, 6], F32, tag="st")
                for j in range(8):
                    nc.vector.bn_stats(
                        out=st[:, j, :], in_=xa[:, j * 512 : (j + 1) * 512]
                    )
                mv = small.tile([128, 2], F32, tag="mv")
                nc.vector.bn_aggr(out=mv[:], in_=st[:])
                sc = small.tile([128, 2], F32, tag="sc")  # [negmu, den]
                nc.scalar.activation(
                    out=sc[:, 1:2],
                    in_=mv[:, 1:2],
                    func=mybir.ActivationFunctionType.Sqrt,
                    scale=float(D) / float(D - 1),
                )
                nc.vector.tensor_scalar_add(sc[:, 1:2], sc[:, 1:2], EPS)
                nc.vector.tensor_scalar_mul(sc[:, 0:1], mv[:, 0:1], -1.0)
                nc.vector.reciprocal(out=inv_all[:, m : m + 1], in_=sc[:, 1:2])
                nc.vector.tensor_mul(
                    a_all[:, m : m + 1], sc[:, 0:1], inv_all[:, m : m + 1]
                )
                # --- transpose x chunk (fp32 PE transposes; 1 DVE wait each) ---
                for k in range(KT):
                    pt = tp_psum.tile([128, 128], F32, tag="tp")
                    nc.tensor.transpose(
                        pt[:], xa[:, k * 128 : (k + 1) * 128], identity[:]
                    )
                    dst = xt[:, k, mc_i * 128 : (mc_i + 1) * 128]
                    nc.vector.tensor_copy(out=dst, in_=pt[:])
                    if mode == "split":
                        hi32 = tmp_pool.tile([128, 128], F32, tag="hi32")
                        nc.vector.tensor_copy(out=hi32[:], in_=dst)
                        nc.vector.tensor_sub(
                            xt_lo[:, k, mc_i * 128 : (mc_i + 1) * 128],
                            pt[:],
                            hi32[:],
                        )

            # --- matmuls against streamed transposed weights ---
            for nw in range(NWCH):
                wt = wst_pool.tile([128, KT, WN], wdt, tag="wst")
                rd = nc.gpsimd.dma_start(out=wt[:], in_=wtq[nw][:, :, :])
                add_dep_helper(
                    rd.ins, w_last_write[nw].ins, sync=True, reason="wtq RAW"
                )
                if mode == "f32r":
                    touch(wt[:, 0, :].bitcast(BF16)[:, 0:128])
                else:
                    touch(wt[:, 0, 0:128])
                if s == 0:
                    # S row: ones-matmul, stage out, bounce via DRAM, broadcast
                    ps_s = s_psum.tile([1, WN], F32, tag="s_ps")
                    for k in range(KT):
                        nc.tensor.matmul(
                            ps_s[:],
                            ones_w[:],
                            wt[:, k, :],
                            start=(k == 0),
                            stop=(k == KT - 1),
                        )
                    sstage = singles.tile([1, WN], F32, tag="sstage")
                    nc.vector.tensor_copy(out=sstage[:], in_=ps_s[:])
                    wr = nc.gpsimd.dma_start(out=s_d[nw][:], in_=sstage[:])
                    sap = s_d[nw][:]
                    br = nc.gpsimd.dma_start(
                        out=s_bc[:, nw, :],
                        in_=bass.AP(
                            tensor=sap.tensor, offset=sap.offset, ap=[[0, 128]] + sap.ap
                        ),
                    )
                    add_dep_helper(br.ins, wr.ins, sync=True, reason="Srow RAW")
                for mc_i in range(MPS):
                    m = s * MPS + mc_i
                    pm = mm_psum.tile([128, WN], F32, tag="mm")
                    tsl = slice(mc_i * 128, (mc_i + 1) * 128)
                    for k in range(KT):
                        nc.tensor.matmul(
                            pm[:],
                            xt[:, k, tsl],
                            wt[:, k, :],
                            start=(k == 0),
                            stop=(k == KT - 1 and xt_lo is None),
                        )
                        if xt_lo is not None:
                            nc.tensor.matmul(
                                pm[:],
                                xt_lo[:, k, tsl],
                                wt[:, k, :],
                                start=False,
                                stop=(k == KT - 1),
                            )
                    # evict: ev = pm*inv + a*S + bias  (all DVE)
                    tv = evt_pool.tile([128, WN], F32, tag="tv")
                    nc.vector.tensor_scalar_mul(
                        tv[:], s_bc[:, nw, :], a_all[:, m : m + 1]
                    )
                    ev = ev_pool.tile([128, WN], F32, tag="ev")
                    nc.vector.tensor_scalar_mul(ev[:], pm[:], inv_all[:, m : m + 1])
                    nc.vector.tensor_add(ev[:], ev[:], tv[:])
                    nc.vector.tensor_add(ev[:], ev[:], b_bc[:, nw, :])
                    nc.gpsimd.dma_start(
                        out=out_ext[m * 128 : (m + 1) * 128, nw * WN : (nw + 1) * WN],
                        in_=ev[:],
                    )
    nc.compile()
    return nc


_NC_CACHE = {}
LAST_RESULTS = None


def kernel(x, weight, bias):
    global LAST_RESULTS
    x = np.ascontiguousarray(np.asarray(x, dtype=np.float32))
    weight = np.asarray(weight, dtype=np.float32)
    bias = np.asarray(bias, dtype=np.float32)

    mode = MODE
    if mode not in _NC_CACHE:
        _NC_CACHE[mode] = _build(mode)
    nc = _NC_CACHE[mode]

    in_maps = []
    for i in range(NCORES):
        in_maps.append(
            {
                "x": x,
                "w": np.ascontiguousarray(weight[i * O : (i + 1) * O]),
                "b": np.ascontiguousarray(bias[i * O : (i + 1) * O]),
            }
        )
    trace = os.environ.get("BITLIN_TRACE", "0") == "1"
    try:
        res = run_bass_kernel_spmd(nc, in_maps, list(range(NCORES)), trace=trace)
    except Exception:
        if not trace:
            raise
        res = run_bass_kernel_spmd(nc, in_maps, list(range(NCORES)), trace=False)
    LAST_RESULTS = res
    return np.concatenate([res.results[i]["out"] for i in range(NCORES)], axis=1)



# revision 2
# speedup vs baseline: 1.2095x; 1.2095x over previous
"""BitLinear TRN2 kernel: out = layernorm(x) @ sign(w).T + bias.

Tensor-parallel over out_features on 8 cores: kernel() takes FULL inputs,
shards w/bias row-wise per core (x replicated), runs the Bass kernel via
run_bass_kernel_spmd, and concats the per-core [8192,2048] out shards.

Design (measured on HW, For_i-amplified timing: ~1.84ms/core):
- Software-pipelined main loop: superblock s+1's x loads, bf16 casts,
  stats, xbar transposes and fp8 casts are all emitted BEFORE the
  matmul chains of superblock s, so the DVE/DMA prefetch work never
  sits behind the chains' evictions in queue order.
- Per-chunk SBUF->SBUF xbar transpose into a contiguous staging tile
  (dst contiguity is required by the xbar), then engine copies split
  the staging into the bf16 k-tiles (xt_bf) and the fp8 k-tiles (xt8,
  cast in the copy). No x DRAM bounce (BITLIN4_XBAR=dram restores it).
- 6 PSUM mm banks so chains can run ~6 deep before an eviction is due.
- bf16 output (host upcasts); KF8=3072 default (measured 1.81e-2).
"""

import os
from contextlib import ExitStack

import numpy as np

import concourse.bass as bass
import concourse.tile as tile
from concourse import bacc
from concourse import mybir
from concourse.bass_utils import run_bass_kernel_spmd
from concourse.tile_rust import add_dep_helper

F32 = mybir.dt.float32
BF16 = mybir.dt.bfloat16
FP8 = mybir.dt.float8e4

T, D, O_FULL, NCORES = 8192, 4096, 16384, 8
O = O_FULL // NCORES
EPS = 1e-5

KT = D // 128
MC = T // 128
WN = 512
NWCH = O // WN
SUP = 512
MPS = SUP // 128
NSUP = T // SUP

KF8 = int(os.environ.get("BITLIN4_KF8", "3072"))  # fp8 share of the contraction
OUT_BF16 = os.environ.get("BITLIN4_OUT", "bf16") == "bf16"
XBAR_SBUF = os.environ.get("BITLIN4_XBAR", "sbuf") == "sbuf"
REPS = int(os.environ.get("BITLIN4_REPS", "1"))

MULT = mybir.AluOpType.mult
ADD = mybir.AluOpType.add
DR = mybir.MatmulPerfMode.DoubleRow

# (kf8, out_bf16, reps, xbar_sbuf) used by the graded kernel() entry
BUILD_CFG = (KF8, OUT_BF16, 1, XBAR_SBUF)


def _build(kf8=KF8, out_bf16=OUT_BF16, reps=REPS, xbar_sbuf=XBAR_SBUF):
    kft = kf8 // 128
    kbt = KT - kft
    assert kft % 2 == 0
    odt = BF16 if out_bf16 else F32

    nc = bacc.Bacc("TRN2", target_bir_lowering=False, debug=False)
    x_ext = nc.declare_dram_parameter("x", [T, D], F32, isOutput=False)
    w_ext = nc.declare_dram_parameter("w", [O, D], F32, isOutput=False)
    b_ext = nc.declare_dram_parameter("b", [O], F32, isOutput=False)
    out_ext = nc.declare_dram_parameter("out", [T, O], odt, isOutput=True)
    s_dram = nc.dram_tensor("srow", [O], F32)
    wsdram = None if xbar_sbuf else nc.dram_tensor("wsd", [O, D], BF16)
    xbdram = None if xbar_sbuf else nc.dram_tensor("xbd", [T, D], BF16)

    with tile.TileContext(nc) as tc, ExitStack() as ctx:
        if reps > 1:
            ctx.enter_context(tc.For_i(0, reps))
        singles = ctx.enter_context(tc.tile_pool(name="singles", bufs=1))
        xa_pool = ctx.enter_context(
            tc.tile_pool(name="xa", bufs=2 if xbar_sbuf else 1)
        )
        xab_pool = ctx.enter_context(tc.tile_pool(name="xab", bufs=2))
        stg_pool = ctx.enter_context(
            tc.tile_pool(name="stg", bufs=2 if xbar_sbuf else 1)
        )
        xtb_pool = ctx.enter_context(tc.tile_pool(name="xtb", bufs=2))
        f8_pool = ctx.enter_context(tc.tile_pool(name="f8", bufs=2))
        wt_pool = ctx.enter_context(tc.tile_pool(name="wt", bufs=1))
        corr_pool = ctx.enter_context(tc.tile_pool(name="corr", bufs=2))
        ev_pool = ctx.enter_context(tc.tile_pool(name="ev", bufs=4))
        small = ctx.enter_context(tc.tile_pool(name="small", bufs=4))
        mm_psum = ctx.enter_context(tc.tile_pool(name="mm_ps", bufs=6, space="PSUM"))
        s_psum = ctx.enter_context(tc.tile_pool(name="s_ps", bufs=1, space="PSUM"))

        ones_w = singles.tile([128, 1], FP8)
        nc.vector.memset(ones_w[:], 1.0)
        inv_all = singles.tile([128, MC], F32)
        a_all = singles.tile([128, MC], F32)
        s_bc = singles.tile([128, O], BF16)
        b_bc = singles.tile([128, O], BF16)

        # bias: broadcast-load f32 into a scratch xa tile, cast to bf16
        bap = b_ext[:]
        btmp = xa_pool.tile([128, D], F32, tag="xa")
        nc.gpsimd.dma_start(
            out=btmp[:, :O],
            in_=bass.AP(tensor=bap.tensor, offset=bap.offset, ap=[[0, 128]] + bap.ap),
        )
        nc.vector.tensor_copy(out=b_bc[:], in_=btmp[:, :O])

        # ---- weight prep: sign -> bf16 -> xbar-transpose -> fp8 resident ----
        wt = wt_pool.tile([128, KT, O], FP8, tag="wt")
        if xbar_sbuf:
            for ob in range(O // 128):
                wa = xa_pool.tile([128, D], F32, tag="xa")
                nc.gpsimd.dma_start(
                    out=wa[:], in_=w_ext[ob * 128 : (ob + 1) * 128, :]
                )
                ws = xab_pool.tile([128, D], BF16, tag="xab")
                nc.scalar.sign(out=ws[:], in_=wa[:])
                stg = stg_pool.tile([128, KT, 128], BF16, tag="stg")
                nc.sync.dma_start_transpose(stg[:], ws[:, :])
                nc.vector.tensor_copy(
                    out=wt[:, :, ob * 128 : (ob + 1) * 128], in_=stg[:]
                )
        else:
            ws_writes = []
            for ob in range(O // 128):
                wa = xa_pool.tile([128, D], F32, tag="xa")
                nc.gpsimd.dma_start(
                    out=wa[:], in_=w_ext[ob * 128 : (ob + 1) * 128, :]
                )
                ws = xab_pool.tile([128, D], BF16, tag="xab")
                nc.scalar.sign(out=ws[:], in_=wa[:])
                wr = nc.gpsimd.dma_start(
                    out=wsdram[ob * 128 : (ob + 1) * 128, :], in_=ws[:]
                )
                ws_writes.append(wr)
            for oc in range(NWCH):
                wstage = stg_pool.tile([128, KT, SUP], BF16, tag="stg_big")
                tr = nc.sync.dma_start_transpose(
                    wstage[:], wsdram[oc * 512 : (oc + 1) * 512, :]
                )
                for j in range(4):
                    add_dep_helper(
                        tr.ins, ws_writes[oc * 4 + j].ins, sync=True, reason="ws RAW"
                    )
                nc.vector.tensor_copy(
                    out=wt[:, :, oc * 512 : (oc + 1) * 512], in_=wstage[:]
                )

        # ---- S row ----
        s_writes = []
        for oc in range(NWCH):
            ps_s = s_psum.tile([1, WN], F32, tag="s_ps")
            for k in range(KT):
                nc.tensor.matmul(
                    ps_s[:],
                    ones_w[:],
                    wt[:, k, oc * WN : (oc + 1) * WN],
                    start=(k == 0),
                    stop=(k == KT - 1),
                )
            srow = small.tile([1, WN], F32, tag="srow")
            nc.vector.tensor_copy(out=srow[:], in_=ps_s[:])
            sw = nc.gpsimd.dma_start(
                out=s_dram[oc * WN : (oc + 1) * WN], in_=srow[:]
            )
            s_writes.append(sw)
        s32 = xa_pool.tile([128, D], F32, tag="xa")
        sap = s_dram[:]
        br = nc.gpsimd.dma_start(
            out=s32[:, :O],
            in_=bass.AP(tensor=sap.tensor, offset=sap.offset, ap=[[0, 128]] + sap.ap),
        )
        for sw in s_writes:
            add_dep_helper(br.ins, sw.ins, sync=True, reason="Srow RAW")
        nc.vector.tensor_copy(out=s_bc[:], in_=s32[:, :O])

        # ---------------- main: software-pipelined superblocks ------------
        xt_cur = [None, None]  # (xt_bf, xt8) for superblock being chained

        def prefetch(s):
            xt_bf = xtb_pool.tile([128, kbt, SUP], BF16, tag="xtbf")
            xt8 = f8_pool.tile([128, kft, SUP], FP8, tag="xt8")
            xab_writes = []
            for mc_i in range(MPS):
                m = s * MPS + mc_i
                xa = xa_pool.tile([128, D], F32, tag="xa")
                nc.gpsimd.dma_start(out=xa[:], in_=x_ext[m * 128 : (m + 1) * 128, :])
                xab = xab_pool.tile([128, D], BF16, tag="xab")
                nc.scalar.copy(out=xab[:], in_=xa[:])
                st = small.tile([128, 8, 6], F32, tag="st")
                for j in range(8):
                    nc.vector.bn_stats(
                        out=st[:, j, :], in_=xab[:, j * 512 : (j + 1) * 512]
                    )
                mv = small.tile([128, 2], F32, tag="mv")
                nc.vector.bn_aggr(out=mv[:], in_=st[:])
                sc = small.tile([128, 2], F32, tag="sc")
                nc.scalar.activation(
                    out=sc[:, 1:2],
                    in_=mv[:, 1:2],
                    func=mybir.ActivationFunctionType.Sqrt,
                    scale=float(D) / float(D - 1),
                )
                nc.vector.tensor_scalar_add(sc[:, 1:2], sc[:, 1:2], EPS)
                nc.vector.tensor_scalar_mul(sc[:, 0:1], mv[:, 0:1], -1.0)
                nc.vector.reciprocal(out=inv_all[:, m : m + 1], in_=sc[:, 1:2])
                nc.vector.tensor_mul(
                    a_all[:, m : m + 1], sc[:, 0:1], inv_all[:, m : m + 1]
                )
                if xbar_sbuf:
                    stg = stg_pool.tile([128, KT, 128], BF16, tag="stg")
                    nc.sync.dma_start_transpose(stg[:], xab[:, :])
                    tsl = slice(mc_i * 128, (mc_i + 1) * 128)
                    nc.vector.tensor_copy(out=xt_bf[:, :, tsl], in_=stg[:, :kbt, :])
                    nc.vector.tensor_copy(out=xt8[:, :, tsl], in_=stg[:, kbt:, :])
                else:
                    xw = nc.gpsimd.dma_start(
                        out=xbdram[m * 128 : (m + 1) * 128, :], in_=xab[:]
                    )
                    xab_writes.append(xw)
            if not xbar_sbuf:
                stg = stg_pool.tile([128, KT, SUP], BF16, tag="stg_big")
                tr = nc.sync.dma_start_transpose(
                    stg[:], xbdram[s * SUP : (s + 1) * SUP, :]
                )
                for xw in xab_writes:
                    add_dep_helper(tr.ins, xw.ins, sync=True, reason="xb RAW")
                nc.vector.tensor_copy(out=xt_bf[:], in_=stg[:, :kbt, :])
                nc.vector.tensor_copy(out=xt8[:], in_=stg[:, kbt:, :])
            return xt_bf, xt8

        def chains(s, xt_bf, xt8):
            for mc_i in range(MPS):
                m = s * MPS + mc_i
                tsl = slice(mc_i * 128, (mc_i + 1) * 128)
                corr = corr_pool.tile([128, O], BF16, tag="corr")
                nc.vector.scalar_tensor_tensor(
                    out=corr[:],
                    in0=s_bc[:],
                    scalar=a_all[:, m : m + 1],
                    in1=b_bc[:],
                    op0=MULT,
                    op1=ADD,
                )
                for nw in range(NWCH):
                    nsl = slice(nw * WN, (nw + 1) * WN)
                    pm = mm_psum.tile([128, WN], F32, tag="mm")
                    for k in range(kbt):
                        nc.tensor.matmul(
                            pm[:],
                            xt_bf[:, k, tsl],
                            wt[:, k, nsl],
                            start=(k == 0),
                            stop=False,
                        )
                    npair = kft // 2
                    for kp in range(npair):
                        nc.tensor.matmul(
                            pm[:],
                            xt8[:, 2 * kp : 2 * kp + 2, tsl],
                            wt[:, kbt + 2 * kp : kbt + 2 * kp + 2, nsl],
                            start=False,
                            stop=(kp == npair - 1),
                            perf_mode=DR,
                        )
                    ev = ev_pool.tile([128, WN], odt, tag="ev")
                    nc.vector.scalar_tensor_tensor(
                        out=ev[:],
                        in0=pm[:],
                        scalar=inv_all[:, m : m + 1],
                        in1=corr[:, nsl],
                        op0=MULT,
                        op1=ADD,
                    )
                    nc.gpsimd.dma_start(
                        out=out_ext[m * 128 : (m + 1) * 128, nsl], in_=ev[:]
                    )

        nxt = prefetch(0)
        for s in range(NSUP):
            cur, nxt = nxt, (prefetch(s + 1) if s + 1 < NSUP else None)
            chains(s, *cur)
    nc.compile()
    return nc


_NC_CACHE = {}
LAST_RESULTS = None


def kernel(x, weight, bias):
    global LAST_RESULTS
    x = np.ascontiguousarray(np.asarray(x, dtype=np.float32))
    weight = np.asarray(weight, dtype=np.float32)
    bias = np.asarray(bias, dtype=np.float32)

    key = (KF8, OUT_BF16, 1, XBAR_SBUF)
    if key not in _NC_CACHE:
        _NC_CACHE[key] = _build(*key)
    nc = _NC_CACHE[key]

    in_maps = []
    for i in range(NCORES):
        in_maps.append(
            {
                "x": x,
                "w": np.ascontiguousarray(weight[i * O : (i + 1) * O]),
                "b": np.ascontiguousarray(bias[i * O : (i + 1) * O]),
            }
        )
    res = run_bass_kernel_spmd(nc, in_maps, list(range(NCORES)), trace=False)
    LAST_RESULTS = res
    out = np.concatenate([res.results[i]["out"] for i in range(NCORES)], axis=1)
    return np.ascontiguousarray(out.astype(np.float32))
